# revision 6
# baseline (speedup 1.0000x reference)
"""Trainium2 Bass kernel for nn_DecoderMemNN (MemN2N decoder).

Strategy (8 NeuronCores, SPMD single NEFF):
 - Batch-parallel over B=32 (4 batches per core) for the memory-hop loop.
 - Per (batch): dma_gather rows of a host-fused table F=[C0|C1|C2] (3KB rows)
   at story indices; a one-hot "E-matrix" PE matmul sums the T=4 gathered rows
   per memory slot into PSUM (plus the positional-encoding rows via an identity
   matmul), giving m_A tiles [128, 8, 768] (m = 128*jt + p).
 - Hop h logits: DVE mult (m_A slice x broadcast u) + reduce; softmax without
   max-subtraction (logit range is small); o_k via PE matmul with prob as the
   M=1 stationary operand; u_{h+1} = u_h + broadcast(o_k) via ones-matmul.
 - C3 is dead (its o_k is never used by any output) and is never touched.
 - p_vocab: AllGather of o_k(hop0) rows across cores, then a vocab-sharded
   (column-parallel) matmul against W1^T [513, 4000] per core (bias folded in
   as an extra contraction row); host concatenates the 8 shards.
 - GRU + hidden computed on every core (tiny, replicated) in transposed layout.

kernel(**inputs) takes FULL inputs (as reference.setup_inputs()) and returns
(prob_lg, p_vocab, hidden) like reference.reference().
"""

import numpy as np

import concourse.bacc as bacc
import concourse.mybir as mybir
import concourse.tile as tile
from concourse.bass_utils import run_bass_kernel_spmd

# problem constants (hardcoded per the harness contract)
V = 32000
D = 256
M = 1024
B = 32
T = 4
NCORES = 8
BPC = B // NCORES          # batches per core = 4
VS = V // NCORES           # vocab shard = 4000
E3 = 3 * D                 # fused row width = 768
NBN = 8                    # p_vocab column blocks per core
NBW = VS // NBN            # 500 columns per matmul
F32 = mybir.dt.float32
I16 = mybir.dt.int16

# set by test harness to collect a profile
PROFILE = False
LAST_RESULTS = None


def _wrap_idx(idx_flat: np.ndarray) -> np.ndarray:
    """int16 index list -> [128, n/16] wrapped layout (elem i at partition i%16,
    col i//16), replicated across the 8 gpsimd core groups."""
    n = idx_flat.shape[0]
    w = idx_flat.reshape(n // 16, 16).T.astype(np.int16)   # [16, n/16]
    return np.tile(w, (8, 1)).copy()


def _build():
    nc = bacc.Bacc("TRN2", target_bir_lowering=False, debug=False,
                   num_devices=NCORES)

    # ---- DRAM I/O ----
    F_d = nc.dram_tensor("F", [V, E3], F32, kind="ExternalInput")
    pos_d = nc.dram_tensor("pos", [1025, D], F32, kind="ExternalInput")
    w1t_d = nc.dram_tensor("w1t", [513, VS], F32, kind="ExternalInput")
    wih_d = nc.dram_tensor("wih", [384, 3 * D], F32, kind="ExternalInput")
    whh_d = nc.dram_tensor("whh", [384, 3 * D], F32, kind="ExternalInput")
    xq_d = nc.dram_tensor("xq", [384, B], F32, kind="ExternalInput")
    h0_d = nc.dram_tensor("h0", [384, B], F32, kind="ExternalInput")
    em_d = nc.dram_tensor("em", [128, 5, 128], F32, kind="ExternalInput")
    sel_d = nc.dram_tensor("sel", [32, BPC, 128], F32, kind="ExternalInput")
    onesr_d = nc.dram_tensor("onesr", [1, 128], F32, kind="ExternalInput")
    ones32_d = nc.dram_tensor("ones32", [1, 32], F32, kind="ExternalInput")
    onescol_d = nc.dram_tensor("onescol", [128, 1], F32, kind="ExternalInput")
    idxg_d = nc.dram_tensor("idxg", [128, BPC, 4, 64], I16, kind="ExternalInput")
    idxp_d = nc.dram_tensor("idxp", [128, BPC, 64], I16, kind="ExternalInput")

    plg_d = nc.dram_tensor("prob_lg", [BPC, M], F32, kind="ExternalOutput")
    pv_d = nc.dram_tensor("p_vocab", [B, VS], F32, kind="ExternalOutput")
    hid_d = nc.dram_tensor("hidden", [B, D], F32, kind="ExternalOutput")

    ADD = mybir.AluOpType.add
    MUL = mybir.AluOpType.mult
    SUB = mybir.AluOpType.subtract
    AX = mybir.AxisListType.X
    SIG = mybir.ActivationFunctionType.Sigmoid
    TANH = mybir.ActivationFunctionType.Tanh
    EXP = mybir.ActivationFunctionType.Exp

    with tile.TileContext(nc) as tc:
        with (
            tc.tile_pool(name="const", bufs=1) as cpool,
            tc.tile_pool(name="work", bufs=1) as wpool,
            tc.tile_pool(name="gbuf", bufs=2) as gpool,
            tc.tile_pool(name="mabuf", bufs=2) as mapool,
            tc.tile_pool(name="small", bufs=2) as spool,
            tc.tile_pool(name="psA", bufs=2, space="PSUM") as psA,
            tc.tile_pool(name="psB", bufs=2, space="PSUM") as psB,
            tc.tile_pool(name="psC", bufs=2, space="PSUM") as psC,
            tc.tile_pool(name="dram", bufs=1, space="DRAM") as dpool,
        ):
            # ---- load constants ----
            em_t = cpool.tile([128, 5, 128], F32)
            nc.sync.dma_start(em_t[:], em_d[:])
            sel_t = cpool.tile([32, BPC, 128], F32)
            nc.sync.dma_start(sel_t[:], sel_d[:])
            onesr_t = cpool.tile([1, 128], F32)
            nc.sync.dma_start(onesr_t[:], onesr_d[:])
            ones32_t = cpool.tile([1, 32], F32)
            nc.sync.dma_start(ones32_t[:], ones32_d[:])
            onescol_t = cpool.tile([128, 1], F32)
            nc.sync.dma_start(onescol_t[:], onescol_d[:])
            idxg_t = cpool.tile([128, BPC, 4, 64], I16)
            nc.sync.dma_start(idxg_t[:], idxg_d[:])
            idxp_t = cpool.tile([128, BPC, 64], I16)
            nc.sync.dma_start(idxp_t[:], idxp_d[:])
            wih_t = cpool.tile([128, 3, 3 * D], F32)
            nc.sync.dma_start(wih_t[:], wih_d[:].rearrange("(c p) e -> p c e", p=128))
            whh_t = cpool.tile([128, 3, 3 * D], F32)
            nc.sync.dma_start(whh_t[:], whh_d[:].rearrange("(c p) e -> p c e", p=128))
            xq_t = cpool.tile([128, 3, B], F32)
            nc.sync.dma_start(xq_t[:], xq_d[:].rearrange("(c p) b -> p c b", p=128))
            h0_t = cpool.tile([128, 3, B], F32)
            nc.sync.dma_start(h0_t[:], h0_d[:].rearrange("(c p) b -> p c b", p=128))

            ident = em_t[:, 4, :]

            # ---- GRU (transposed layout; all 32 batches) ----
            gi_ps = psC.tile([128, 6, B], F32, tag="pvps")
            gh_ps = psC.tile([128, 6, B], F32, tag="pvps")
            for ps, w_t, x_t in ((gi_ps, wih_t, xq_t), (gh_ps, whh_t, h0_t)):
                for mc in range(6):
                    for kc in range(3):
                        kk = 128 if kc < 2 else 1
                        nc.tensor.matmul(
                            ps[:, mc, :],
                            w_t[0:kk, kc, mc * 128:(mc + 1) * 128],
                            x_t[0:kk, kc, :],
                            start=(kc == 0), stop=(kc == 2),
                        )
            gi_t = wpool.tile([128, 6, B], F32)
            nc.scalar.copy(gi_t[:], gi_ps[:])
            rz_t = wpool.tile([128, 4, B], F32)
            nc.vector.tensor_tensor(rz_t[:], gi_t[:, 0:4, :], gh_ps[:, 0:4, :], ADD)
            r_t = wpool.tile([128, 2, B], F32)
            nc.scalar.activation(r_t[:], rz_t[:, 0:2, :], SIG)
            z_t = wpool.tile([128, 2, B], F32)
            nc.scalar.activation(z_t[:], rz_t[:, 2:4, :], SIG)
            t3_t = wpool.tile([128, 2, B], F32)
            nc.vector.tensor_tensor(t3_t[:], r_t[:], gh_ps[:, 4:6, :], MUL)
            t4_t = wpool.tile([128, 2, B], F32)
            nc.vector.tensor_tensor(t4_t[:], t3_t[:], gi_t[:, 4:6, :], ADD)
            n_t = wpool.tile([128, 2, B], F32)
            nc.scalar.activation(n_t[:], t4_t[:], TANH)
            d_t = wpool.tile([128, 2, B], F32)
            nc.vector.tensor_tensor(d_t[:], h0_t[:, 0:2, :], n_t[:], SUB)
            zd_t = wpool.tile([128, 2, B], F32)
            nc.vector.tensor_tensor(zd_t[:], z_t[:], d_t[:], MUL)
            hn_t = wpool.tile([128, 2, B], F32)   # h_new^T = u0^T, all 32 batches
            nc.vector.tensor_tensor(hn_t[:], n_t[:], zd_t[:], ADD)

            # h_new rows [32, 256] (hidden output + u0 broadcast source)
            hr_ps = psB.tile([32, 2, 128], F32, tag="sps")
            for c in range(2):
                nc.tensor.matmul(hr_ps[:, c, :], hn_t[:, c, :], ident,
                                 start=True, stop=True)
            hr_t = wpool.tile([32, 2, 128], F32)
            nc.scalar.copy(hr_t[:], hr_ps[:])
            nc.sync.dma_start(hid_d[:], hr_t[:].rearrange("b c p -> b (c p)"))
            hrows = hr_t[:].rearrange("b c p -> b (c p)")  # [32, 256]

            okin_t = dpool.tile([BPC, D], F32)
            okall_t = dpool.tile([NCORES, BPC, D], F32)

            u_tiles = []
            # ---- per local batch ----
            for i in range(BPC):
                # u0 broadcast: sel_i^T @ hrows -> [128, 256]
                ub_ps = psB.tile([128, D], F32, tag="sps")
                nc.tensor.matmul(ub_ps[:], sel_t[:, i, :], hrows,
                                 start=True, stop=True)
                u_t = wpool.tile([128, D], F32, tag=f"u{i}")
                nc.scalar.copy(u_t[:], ub_ps[:])
                u_tiles.append(u_t)

                posg_t = spool.tile([128, 8, D], F32, tag="posg")
                nc.gpsimd.dma_gather(posg_t[:], pos_d[:], idxp_t[:, i, :],
                                     M, M, D)

                mA_t = mapool.tile([128, 8, E3], F32, tag="mA")
                for qt in range(4):
                    g_t = gpool.tile([128, 8, E3], F32, tag="G")
                    nc.gpsimd.dma_gather(g_t[:], F_d[:], idxg_t[:, i, qt, :],
                                         M, M, E3)
                    for jj in range(2):
                        jt = qt * 2 + jj
                        ps = psA.tile([128, E3], F32, tag="mAps")
                        for r in range(3):
                            nc.tensor.matmul(
                                ps[:, 0:512], em_t[:, r, :],
                                g_t[:, jj * 4 + r, 0:512],
                                start=(r == 0), stop=False,
                                skip_group_check=True)
                        nc.tensor.matmul(
                            ps[:, 0:D], ident, posg_t[:, jt, :],
                            start=False, stop=False, skip_group_check=True)
                        nc.tensor.matmul(
                            ps[:, 0:512], em_t[:, 3, :],
                            g_t[:, jj * 4 + 3, 0:512],
                            start=False, stop=True, skip_group_check=True)
                        for r in range(4):
                            nc.tensor.matmul(
                                ps[:, 512:E3], em_t[:, r, :],
                                g_t[:, jj * 4 + r, 512:E3],
                                start=(r == 0), stop=(r == 3),
                                skip_group_check=True)
                        nc.scalar.copy(mA_t[:, jt, :], ps[:])

                # ---- hops ----
                for h in range(3):
                    prod_t = spool.tile([128, 8, D], F32, tag="prod")
                    u_b = u_t[:].rearrange("p (o d) -> p o d", o=1) \
                        .broadcast_to([128, 8, D])
                    nc.vector.tensor_tensor(
                        prod_t[:], mA_t[:, :, h * D:(h + 1) * D], u_b, MUL)
                    logits_t = spool.tile([128, 8], F32, tag="logits")
                    nc.vector.tensor_reduce(logits_t[:], prod_t[:], AX, ADD)

                    if h == 2:
                        nc.sync.dma_start(
                            plg_d[i].rearrange("(j p) -> p j", p=128),
                            logits_t[:])
                        continue

                    exp_t = spool.tile([128, 8], F32, tag="exp")
                    rsum_t = spool.tile([128, 1], F32, tag="rsum")
                    nc.scalar.activation(exp_t[:], logits_t[:], EXP,
                                         accum_out=rsum_t[:])
                    tot_ps = psB.tile([1, 1], F32, tag="sps")
                    nc.tensor.matmul(tot_ps[:], rsum_t[:], onescol_t[:],
                                     start=True, stop=True)
                    tot_t = spool.tile([1, 1], F32, tag="tot")
                    nc.scalar.copy(tot_t[:], tot_ps[:])
                    totb_ps = psB.tile([128, 1], F32, tag="sps")
                    nc.tensor.matmul(totb_ps[:], onesr_t[:], tot_t[:],
                                     start=True, stop=True)
                    rinv_t = spool.tile([128, 1], F32, tag="rinv")
                    nc.vector.reciprocal(rinv_t[:], totb_ps[:])
                    prob_t = spool.tile([128, 8], F32, tag="prob")
                    nc.vector.tensor_scalar_mul(prob_t[:], exp_t[:], rinv_t[:])

                    ok_ps = psB.tile([1, D], F32, tag="sps")
                    for jt in range(8):
                        nc.tensor.matmul(
                            ok_ps[:], prob_t[:, jt:jt + 1],
                            mA_t[:, jt, (h + 1) * D:(h + 2) * D],
                            start=(jt == 0), stop=(jt == 7))
                    ok_t = spool.tile([1, D], F32, tag="ok")
                    nc.scalar.copy(ok_t[:], ok_ps[:])
                    if h == 0:
                        nc.sync.dma_start(okin_t[i:i + 1, :], ok_t[:])
                    ub2_ps = psB.tile([128, D], F32, tag="sps")
                    nc.tensor.matmul(ub2_ps[:], onesr_t[:], ok_t[:],
                                     start=True, stop=True)
                    nc.vector.tensor_tensor(u_t[:], u_t[:], ub2_ps[:], ADD)

            # ---- AllGather o_k(hop0) rows; build x^T; W1 projection ----
            nc.gpsimd.collective_compute(
                "AllGather", mybir.AluOpType.bypass,
                replica_groups=[list(range(NCORES))],
                ins=[okin_t.opt()], outs=[okall_t.opt()])
            okrows_t = wpool.tile([32, D], F32)
            nc.sync.dma_start(okrows_t[:],
                              okall_t[:].rearrange("k i d -> (k i) d"))
            okT_ps = psB.tile([128, 2, 32], F32, tag="sps")
            for c in range(2):
                nc.tensor.matmul(okT_ps[:, c, :],
                                 okrows_t[:, c * 128:(c + 1) * 128],
                                 ident[0:32, 0:32], start=True, stop=True)
            okT_t = wpool.tile([128, 2, 32], F32)
            nc.scalar.copy(okT_t[:], okT_ps[:])

            for nb in range(NBN):
                wk_t = spool.tile([128, 4, NBW], F32, tag="wk")
                nc.sync.dma_start(
                    wk_t[:],
                    w1t_d[0:512, nb * NBW:(nb + 1) * NBW]
                    .rearrange("(c p) n -> p c n", p=128))
                wb_t = spool.tile([1, NBW], F32, tag="wb")
                nc.sync.dma_start(wb_t[:], w1t_d[512:513, nb * NBW:(nb + 1) * NBW])
                pv_ps = psC.tile([32, NBW], F32, tag="pvps")
                for kc in range(2):
                    nc.tensor.matmul(pv_ps[:], hn_t[:, kc, :], wk_t[:, kc, :],
                                     start=(kc == 0), stop=False)
                for kc in range(2):
                    nc.tensor.matmul(pv_ps[:], okT_t[:, kc, :],
                                     wk_t[:, kc + 2, :],
                                     start=False, stop=False)
                nc.tensor.matmul(pv_ps[:], ones32_t[:], wb_t[:],
                                 start=False, stop=True)
                pv_t = spool.tile([32, NBW], F32, tag="pv")
                nc.scalar.copy(pv_t[:], pv_ps[:])
                nc.sync.dma_start(pv_d[:, nb * NBW:(nb + 1) * NBW], pv_t[:])

    nc.compile()
    return nc


_NC_CACHE = None


def prepare_in_maps(story, enc_query, last_hidden, C, pos_table, W1_w, W1_b,
                    gru_w_ih, gru_w_hh, gru_b_ih, gru_b_hh):
    story = np.asarray(story)
    enc_query = np.asarray(enc_query)
    last_hidden = np.asarray(last_hidden, dtype=np.float32)
    C = np.asarray(C, dtype=np.float32)
    pos_table = np.asarray(pos_table, dtype=np.float32)
    W1_w = np.asarray(W1_w, dtype=np.float32)
    W1_b = np.asarray(W1_b, dtype=np.float32)
    gru_w_ih = np.asarray(gru_w_ih, dtype=np.float32)
    gru_w_hh = np.asarray(gru_w_hh, dtype=np.float32)
    gru_b_ih = np.asarray(gru_b_ih, dtype=np.float32)
    gru_b_hh = np.asarray(gru_b_hh, dtype=np.float32)

    # ---- host-side index/layout prep (integer work + small reshapes) ----
    s = np.ascontiguousarray(story.transpose(1, 0, 2))       # [B, M, T] int64
    pad_mask = s[:, :, 0] == 0
    positions = np.cumsum(~pad_mask, axis=-1).astype(np.int64)
    positions[pad_mask] = 0                                   # [B, M]

    F = np.ascontiguousarray(
        np.concatenate([C[0], C[1], C[2]], axis=1), dtype=np.float32)

    # augmented GRU operands (bias folded as contraction row 256; pad to 384)
    def aug_w(wT, b):
        out = np.zeros((384, 3 * D), dtype=np.float32)
        out[0:D] = wT
        out[D] = b
        return out

    wihA = aug_w(gru_w_ih.T, gru_b_ih)
    whhA = aug_w(gru_w_hh.T, gru_b_hh)

    def aug_x(xT):
        out = np.zeros((384, B), dtype=np.float32)
        out[0:D] = xT
        out[D] = 1.0
        return out

    xqA = aug_x(C[0][enc_query].T)
    h0A = aug_x(last_hidden[0].T)

    em = np.zeros((128, 5, 128), dtype=np.float32)
    for q in range(4):
        for p in range(128):
            em[p, q, 32 * q + p // 4] = 1.0
    em[:, 4, :] = np.eye(128, dtype=np.float32)

    onesr = np.ones((1, 128), dtype=np.float32)
    ones32 = np.ones((1, 32), dtype=np.float32)
    onescol = np.ones((128, 1), dtype=np.float32)

    w1t_full = np.concatenate([W1_w.T, W1_b[None, :]], axis=0)  # [513, 32000]

    in_maps = []
    for k in range(NCORES):
        sel = np.zeros((32, BPC, 128), dtype=np.float32)
        for i in range(BPC):
            sel[BPC * k + i, i, :] = 1.0
        idxg = np.zeros((128, BPC, 4, 64), dtype=np.int16)
        idxp = np.zeros((128, BPC, 64), dtype=np.int16)
        for i in range(BPC):
            b = BPC * k + i
            flat = s[b].reshape(-1).astype(np.int16)          # j = m*4+t
            for qt in range(4):
                idxg[:, i, qt, :] = _wrap_idx(flat[qt * M:(qt + 1) * M])
            idxp[:, i, :] = _wrap_idx(positions[b].astype(np.int16))
        in_maps.append({
            "F": F,
            "pos": pos_table,
            "w1t": np.ascontiguousarray(w1t_full[:, k * VS:(k + 1) * VS]),
            "wih": wihA, "whh": whhA, "xq": xqA, "h0": h0A,
            "em": em, "sel": sel,
            "onesr": onesr, "ones32": ones32, "onescol": onescol,
            "idxg": idxg, "idxp": idxp,
        })
    return in_maps


def kernel(**inputs):
    global _NC_CACHE, LAST_RESULTS
    if _NC_CACHE is None:
        _NC_CACHE = _build()
    nc = _NC_CACHE

    in_maps = prepare_in_maps(**inputs)
    res = run_bass_kernel_spmd(nc, in_maps, core_ids=list(range(NCORES)),
                               trace=PROFILE)
    LAST_RESULTS = res

    prob_lg = np.concatenate([np.asarray(res.results[k]["prob_lg"])
                              for k in range(NCORES)], axis=0)
    p_vocab = np.concatenate([np.asarray(res.results[k]["p_vocab"])
                              for k in range(NCORES)], axis=1)
    hidden = np.asarray(res.results[0]["hidden"])[None]
    return prob_lg.astype(np.float32), p_vocab.astype(np.float32), \
        hidden.astype(np.float32)


# revision 10
# speedup vs baseline: 1.0134x; 1.0134x over previous
"""Trainium2 Bass kernel for nn_DecoderMemNN (MemN2N decoder).

Strategy (8 NeuronCores, SPMD single NEFF):
 - Batch-parallel over B=32 (4 batches per core) for the memory-hop loop.
 - Per (batch): dma_gather rows of a host-fused table F=[C0|C1|C2] (3KB rows)
   at story indices; a one-hot "E-matrix" PE matmul sums the T=4 gathered rows
   per memory slot into PSUM (plus the positional-encoding rows via an identity
   matmul), giving m_A tiles [128, 8, 768] (m = 128*jt + p).
 - Hop h logits: DVE mult (m_A slice x broadcast u) + reduce; softmax without
   max-subtraction (logit range is small); o_k via PE matmul with prob as the
   M=1 stationary operand; u_{h+1} = u_h + broadcast(o_k) via ones-matmul.
 - C3 is dead (its o_k is never used by any output) and is never touched.
 - p_vocab: AllGather of o_k(hop0) rows across cores, then a vocab-sharded
   (column-parallel) matmul against W1^T [513, 4000] per core (bias folded in
   as an extra contraction row); host concatenates the 8 shards.
 - GRU + hidden computed on every core (tiny, replicated) in transposed layout.

kernel(**inputs) takes FULL inputs (as reference.setup_inputs()) and returns
(prob_lg, p_vocab, hidden) like reference.reference().
"""

import numpy as np

import concourse.bacc as bacc
import concourse.mybir as mybir
import concourse.tile as tile
from concourse.bass_utils import run_bass_kernel_spmd

# problem constants (hardcoded per the harness contract)
V = 32000
D = 256
M = 1024
B = 32
T = 4
NCORES = 8
BPC = B // NCORES          # batches per core = 4
VS = V // NCORES           # vocab shard = 4000
E3 = 3 * D                 # fused row width = 768
NBN = 8                    # p_vocab column blocks per core
NBW = VS // NBN            # 500 columns per matmul
F32 = mybir.dt.float32
I16 = mybir.dt.int16

# set by test harness to collect a profile
PROFILE = False
LAST_RESULTS = None


def _wrap_idx(idx_flat: np.ndarray) -> np.ndarray:
    """int16 index list -> [128, n/16] wrapped layout (elem i at partition i%16,
    col i//16), replicated across the 8 gpsimd core groups."""
    n = idx_flat.shape[0]
    w = idx_flat.reshape(n // 16, 16).T.astype(np.int16)   # [16, n/16]
    return np.tile(w, (8, 1)).copy()


def _build(num_devices=NCORES, with_collective=True, work_reps=1):
    nc = bacc.Bacc("TRN2", target_bir_lowering=False, debug=False,
                   num_devices=num_devices)

    # ---- DRAM I/O ----
    F_d = nc.dram_tensor("F", [V, E3], F32, kind="ExternalInput")
    pos_d = nc.dram_tensor("pos", [1025, D], F32, kind="ExternalInput")
    w1t_d = nc.dram_tensor("w1t", [513, VS], F32, kind="ExternalInput")
    wih_d = nc.dram_tensor("wih", [384, 3 * D], F32, kind="ExternalInput")
    whh_d = nc.dram_tensor("whh", [384, 3 * D], F32, kind="ExternalInput")
    xq_d = nc.dram_tensor("xq", [384, B], F32, kind="ExternalInput")
    h0_d = nc.dram_tensor("h0", [384, B], F32, kind="ExternalInput")
    em_d = nc.dram_tensor("em", [128, 5, 128], F32, kind="ExternalInput")
    sel_d = nc.dram_tensor("sel", [32, BPC, 128], F32, kind="ExternalInput")
    onesr_d = nc.dram_tensor("onesr", [1, 128], F32, kind="ExternalInput")
    ones32_d = nc.dram_tensor("ones32", [1, 32], F32, kind="ExternalInput")
    onescol_d = nc.dram_tensor("onescol", [128, 1], F32, kind="ExternalInput")
    idxg_d = nc.dram_tensor("idxg", [128, BPC, 4, 64], I16, kind="ExternalInput")
    idxp_d = nc.dram_tensor("idxp", [128, BPC, 64], I16, kind="ExternalInput")

    plg_d = nc.dram_tensor("prob_lg", [BPC, M], F32, kind="ExternalOutput")
    pv_d = nc.dram_tensor("p_vocab", [B, VS], F32, kind="ExternalOutput")
    hid_d = nc.dram_tensor("hidden", [B, D], F32, kind="ExternalOutput")

    ADD = mybir.AluOpType.add
    MUL = mybir.AluOpType.mult
    SUB = mybir.AluOpType.subtract
    AX = mybir.AxisListType.X
    SIG = mybir.ActivationFunctionType.Sigmoid
    TANH = mybir.ActivationFunctionType.Tanh
    EXP = mybir.ActivationFunctionType.Exp

    with tile.TileContext(nc) as tc:
        with (
            tc.tile_pool(name="const", bufs=1) as cpool,
            tc.tile_pool(name="work", bufs=1) as wpool,
            tc.tile_pool(name="gbuf", bufs=2) as gpool,
            tc.tile_pool(name="mabuf", bufs=2) as mapool,
            tc.tile_pool(name="small", bufs=2) as spool,
            tc.tile_pool(name="psA", bufs=2, space="PSUM") as psA,
            tc.tile_pool(name="psB", bufs=2, space="PSUM") as psB,
            tc.tile_pool(name="psC", bufs=2, space="PSUM") as psC,
            tc.tile_pool(name="dram", bufs=1, space="DRAM") as dpool,
        ):
            # ---- load constants ----
            em_t = cpool.tile([128, 5, 128], F32)
            nc.sync.dma_start(em_t[:], em_d[:])
            sel_t = cpool.tile([32, BPC, 128], F32)
            nc.sync.dma_start(sel_t[:], sel_d[:])
            onesr_t = cpool.tile([1, 128], F32)
            nc.sync.dma_start(onesr_t[:], onesr_d[:])
            ones32_t = cpool.tile([1, 32], F32)
            nc.sync.dma_start(ones32_t[:], ones32_d[:])
            onescol_t = cpool.tile([128, 1], F32)
            nc.sync.dma_start(onescol_t[:], onescol_d[:])
            idxg_t = cpool.tile([128, BPC, 4, 64], I16)
            nc.sync.dma_start(idxg_t[:], idxg_d[:])
            idxp_t = cpool.tile([128, BPC, 64], I16)
            nc.sync.dma_start(idxp_t[:], idxp_d[:])
            wih_t = cpool.tile([128, 3, 3 * D], F32)
            nc.sync.dma_start(wih_t[:], wih_d[:].rearrange("(c p) e -> p c e", p=128))
            whh_t = cpool.tile([128, 3, 3 * D], F32)
            nc.sync.dma_start(whh_t[:], whh_d[:].rearrange("(c p) e -> p c e", p=128))
            xq_t = cpool.tile([128, 3, B], F32)
            nc.sync.dma_start(xq_t[:], xq_d[:].rearrange("(c p) b -> p c b", p=128))
            h0_t = cpool.tile([128, 3, B], F32)
            nc.sync.dma_start(h0_t[:], h0_d[:].rearrange("(c p) b -> p c b", p=128))

            ident = em_t[:, 4, :]

            # ---- GRU (transposed layout; all 32 batches) ----
            gi_ps = psC.tile([128, 6, B], F32, tag="pvps")
            gh_ps = psC.tile([128, 6, B], F32, tag="pvps")
            for ps, w_t, x_t in ((gi_ps, wih_t, xq_t), (gh_ps, whh_t, h0_t)):
                for mc in range(6):
                    for kc in range(3):
                        kk = 128 if kc < 2 else 1
                        nc.tensor.matmul(
                            ps[:, mc, :],
                            w_t[0:kk, kc, mc * 128:(mc + 1) * 128],
                            x_t[0:kk, kc, :],
                            start=(kc == 0), stop=(kc == 2),
                        )
            gi_t = wpool.tile([128, 6, B], F32)
            nc.scalar.copy(gi_t[:], gi_ps[:])
            rz_t = wpool.tile([128, 4, B], F32)
            nc.vector.tensor_tensor(rz_t[:], gi_t[:, 0:4, :], gh_ps[:, 0:4, :], ADD)
            r_t = wpool.tile([128, 2, B], F32)
            nc.scalar.activation(r_t[:], rz_t[:, 0:2, :], SIG)
            z_t = wpool.tile([128, 2, B], F32)
            nc.scalar.activation(z_t[:], rz_t[:, 2:4, :], SIG)
            t3_t = wpool.tile([128, 2, B], F32)
            nc.vector.tensor_tensor(t3_t[:], r_t[:], gh_ps[:, 4:6, :], MUL)
            t4_t = wpool.tile([128, 2, B], F32)
            nc.vector.tensor_tensor(t4_t[:], t3_t[:], gi_t[:, 4:6, :], ADD)
            n_t = wpool.tile([128, 2, B], F32)
            nc.scalar.activation(n_t[:], t4_t[:], TANH)
            d_t = wpool.tile([128, 2, B], F32)
            nc.vector.tensor_tensor(d_t[:], h0_t[:, 0:2, :], n_t[:], SUB)
            zd_t = wpool.tile([128, 2, B], F32)
            nc.vector.tensor_tensor(zd_t[:], z_t[:], d_t[:], MUL)
            hn_t = wpool.tile([128, 2, B], F32)   # h_new^T = u0^T, all 32 batches
            nc.vector.tensor_tensor(hn_t[:], n_t[:], zd_t[:], ADD)

            # h_new rows [32, 256] (hidden output + u0 broadcast source)
            hr_ps = psB.tile([32, 2, 128], F32, tag="sps")
            for c in range(2):
                nc.tensor.matmul(hr_ps[:, c, :], hn_t[:, c, :], ident,
                                 start=True, stop=True)
            hr_t = wpool.tile([32, 2, 128], F32)
            nc.scalar.copy(hr_t[:], hr_ps[:])
            nc.sync.dma_start(hid_d[:], hr_t[:].rearrange("b c p -> b (c p)"))
            hrows = hr_t[:].rearrange("b c p -> b (c p)")  # [32, 256]

            okin_t = dpool.tile([BPC, D], F32)
            okall_t = dpool.tile([NCORES, BPC, D], F32)

            u_tiles = []
            # ---- per local batch ----
            for i0 in range(BPC * work_reps):
                i = i0 % BPC
                # u0 broadcast: sel_i^T @ hrows -> [128, 256]
                ub_ps = psB.tile([128, D], F32, tag="sps")
                nc.tensor.matmul(ub_ps[:], sel_t[:, i, :], hrows,
                                 start=True, stop=True)
                u_t = wpool.tile([128, D], F32, tag=f"u{i}")
                nc.scalar.copy(u_t[:], ub_ps[:])
                u_tiles.append(u_t)

                posg_t = spool.tile([128, 8, D], F32, tag="posg")
                nc.gpsimd.dma_gather(posg_t[:], pos_d[:], idxp_t[:, i, :],
                                     M, M, D)

                mA_t = mapool.tile([128, 8, E3], F32, tag="mA")
                for qt in range(4):
                    g_t = gpool.tile([128, 8, E3], F32, tag="G")
                    nc.gpsimd.dma_gather(g_t[:], F_d[:], idxg_t[:, i, qt, :],
                                         M, M, E3)
                    for jj in range(2):
                        jt = qt * 2 + jj
                        ps = psA.tile([128, E3], F32, tag="mAps")
                        for r in range(3):
                            nc.tensor.matmul(
                                ps[:, 0:512], em_t[:, r, :],
                                g_t[:, jj * 4 + r, 0:512],
                                start=(r == 0), stop=False,
                                skip_group_check=True)
                        nc.tensor.matmul(
                            ps[:, 0:D], ident, posg_t[:, jt, :],
                            start=False, stop=False, skip_group_check=True)
                        nc.tensor.matmul(
                            ps[:, 0:512], em_t[:, 3, :],
                            g_t[:, jj * 4 + 3, 0:512],
                            start=False, stop=True, skip_group_check=True)
                        for r in range(4):
                            nc.tensor.matmul(
                                ps[:, 512:E3], em_t[:, r, :],
                                g_t[:, jj * 4 + r, 512:E3],
                                start=(r == 0), stop=(r == 3),
                                skip_group_check=True)
                        nc.scalar.copy(mA_t[:, jt, :], ps[:])

                # ---- hops ----
                for h in range(3):
                    prod_t = spool.tile([128, 8, D], F32, tag="prod")
                    u_b = u_t[:].rearrange("p (o d) -> p o d", o=1) \
                        .broadcast_to([128, 8, D])
                    nc.vector.tensor_tensor(
                        prod_t[:], mA_t[:, :, h * D:(h + 1) * D], u_b, MUL)
                    logits_t = spool.tile([128, 8], F32, tag="logits")
                    nc.vector.tensor_reduce(logits_t[:], prod_t[:], AX, ADD)

                    if h == 2:
                        nc.sync.dma_start(
                            plg_d[i].rearrange("(j p) -> p j", p=128),
                            logits_t[:])
                        continue

                    exp_t = spool.tile([128, 8], F32, tag="exp")
                    rsum_t = spool.tile([128, 1], F32, tag="rsum")
                    nc.scalar.activation(exp_t[:], logits_t[:], EXP,
                                         accum_out=rsum_t[:])
                    tot_ps = psB.tile([1, 1], F32, tag="sps")
                    nc.tensor.matmul(tot_ps[:], rsum_t[:], onescol_t[:],
                                     start=True, stop=True)
                    tot_t = spool.tile([1, 1], F32, tag="tot")
                    nc.scalar.copy(tot_t[:], tot_ps[:])
                    totb_ps = psB.tile([128, 1], F32, tag="sps")
                    nc.tensor.matmul(totb_ps[:], onesr_t[:], tot_t[:],
                                     start=True, stop=True)
                    rinv_t = spool.tile([128, 1], F32, tag="rinv")
                    nc.vector.reciprocal(rinv_t[:], totb_ps[:])
                    prob_t = spool.tile([128, 8], F32, tag="prob")
                    nc.vector.tensor_scalar_mul(prob_t[:], exp_t[:], rinv_t[:])

                    ok_ps = psB.tile([1, D], F32, tag="sps")
                    for jt in range(8):
                        nc.tensor.matmul(
                            ok_ps[:], prob_t[:, jt:jt + 1],
                            mA_t[:, jt, (h + 1) * D:(h + 2) * D],
                            start=(jt == 0), stop=(jt == 7))
                    ok_t = spool.tile([1, D], F32, tag="ok")
                    nc.scalar.copy(ok_t[:], ok_ps[:])
                    if h == 0:
                        nc.sync.dma_start(okin_t[i:i + 1, :], ok_t[:])
                    ub2_ps = psB.tile([128, D], F32, tag="sps")
                    nc.tensor.matmul(ub2_ps[:], onesr_t[:], ok_t[:],
                                     start=True, stop=True)
                    nc.vector.tensor_tensor(u_t[:], u_t[:], ub2_ps[:], ADD)

            # ---- AllGather o_k(hop0) rows; build x^T; W1 projection ----
            if with_collective:
                nc.gpsimd.collective_compute(
                    "AllGather", mybir.AluOpType.bypass,
                    replica_groups=[list(range(num_devices))],
                    ins=[okin_t.opt()], outs=[okall_t.opt()])
            else:
                # timing-model variant: local stand-in for the AllGather
                nc.sync.dma_start(okall_t[0], okin_t[:])
            okrows_t = wpool.tile([32, D], F32)
            nc.sync.dma_start(okrows_t[:],
                              okall_t[:].rearrange("k i d -> (k i) d"))
            okT_ps = psB.tile([128, 2, 32], F32, tag="sps")
            for c in range(2):
                nc.tensor.matmul(okT_ps[:, c, :],
                                 okrows_t[:, c * 128:(c + 1) * 128],
                                 ident[0:32, 0:32], start=True, stop=True)
            okT_t = wpool.tile([128, 2, 32], F32)
            nc.scalar.copy(okT_t[:], okT_ps[:])

            for nb in range(NBN):
                wk_t = spool.tile([128, 4, NBW], F32, tag="wk")
                nc.sync.dma_start(
                    wk_t[:],
                    w1t_d[0:512, nb * NBW:(nb + 1) * NBW]
                    .rearrange("(c p) n -> p c n", p=128))
                wb_t = spool.tile([1, NBW], F32, tag="wb")
                nc.sync.dma_start(wb_t[:], w1t_d[512:513, nb * NBW:(nb + 1) * NBW])
                pv_ps = psC.tile([32, NBW], F32, tag="pvps")
                for kc in range(2):
                    nc.tensor.matmul(pv_ps[:], hn_t[:, kc, :], wk_t[:, kc, :],
                                     start=(kc == 0), stop=False)
                for kc in range(2):
                    nc.tensor.matmul(pv_ps[:], okT_t[:, kc, :],
                                     wk_t[:, kc + 2, :],
                                     start=False, stop=False)
                nc.tensor.matmul(pv_ps[:], ones32_t[:], wb_t[:],
                                 start=False, stop=True)
                pv_t = spool.tile([32, NBW], F32, tag="pv")
                nc.scalar.copy(pv_t[:], pv_ps[:])
                nc.sync.dma_start(pv_d[:, nb * NBW:(nb + 1) * NBW], pv_t[:])

    nc.compile()
    return nc


_NC_CACHE = None


def prepare_in_maps(story, enc_query, last_hidden, C, pos_table, W1_w, W1_b,
                    gru_w_ih, gru_w_hh, gru_b_ih, gru_b_hh):
    story = np.asarray(story)
    enc_query = np.asarray(enc_query)
    last_hidden = np.asarray(last_hidden, dtype=np.float32)
    C = np.asarray(C, dtype=np.float32)
    pos_table = np.asarray(pos_table, dtype=np.float32)
    W1_w = np.asarray(W1_w, dtype=np.float32)
    W1_b = np.asarray(W1_b, dtype=np.float32)
    gru_w_ih = np.asarray(gru_w_ih, dtype=np.float32)
    gru_w_hh = np.asarray(gru_w_hh, dtype=np.float32)
    gru_b_ih = np.asarray(gru_b_ih, dtype=np.float32)
    gru_b_hh = np.asarray(gru_b_hh, dtype=np.float32)

    # ---- host-side index/layout prep (integer work + small reshapes) ----
    s = np.ascontiguousarray(story.transpose(1, 0, 2))       # [B, M, T] int64
    pad_mask = s[:, :, 0] == 0
    positions = np.cumsum(~pad_mask, axis=-1).astype(np.int64)
    positions[pad_mask] = 0                                   # [B, M]

    F = np.ascontiguousarray(
        np.concatenate([C[0], C[1], C[2]], axis=1), dtype=np.float32)

    # augmented GRU operands (bias folded as contraction row 256; pad to 384)
    def aug_w(wT, b):
        out = np.zeros((384, 3 * D), dtype=np.float32)
        out[0:D] = wT
        out[D] = b
        return out

    wihA = aug_w(gru_w_ih.T, gru_b_ih)
    whhA = aug_w(gru_w_hh.T, gru_b_hh)

    def aug_x(xT):
        out = np.zeros((384, B), dtype=np.float32)
        out[0:D] = xT
        out[D] = 1.0
        return out

    xqA = aug_x(C[0][enc_query].T)
    h0A = aug_x(last_hidden[0].T)

    em = np.zeros((128, 5, 128), dtype=np.float32)
    for q in range(4):
        for p in range(128):
            em[p, q, 32 * q + p // 4] = 1.0
    em[:, 4, :] = np.eye(128, dtype=np.float32)

    onesr = np.ones((1, 128), dtype=np.float32)
    ones32 = np.ones((1, 32), dtype=np.float32)
    onescol = np.ones((128, 1), dtype=np.float32)

    w1t_full = np.concatenate([W1_w.T, W1_b[None, :]], axis=0)  # [513, 32000]

    in_maps = []
    for k in range(NCORES):
        sel = np.zeros((32, BPC, 128), dtype=np.float32)
        for i in range(BPC):
            sel[BPC * k + i, i, :] = 1.0
        idxg = np.zeros((128, BPC, 4, 64), dtype=np.int16)
        idxp = np.zeros((128, BPC, 64), dtype=np.int16)
        for i in range(BPC):
            b = BPC * k + i
            flat = s[b].reshape(-1).astype(np.int16)          # j = m*4+t
            for qt in range(4):
                idxg[:, i, qt, :] = _wrap_idx(flat[qt * M:(qt + 1) * M])
            idxp[:, i, :] = _wrap_idx(positions[b].astype(np.int16))
        in_maps.append({
            "F": F,
            "pos": pos_table,
            "w1t": np.ascontiguousarray(w1t_full[:, k * VS:(k + 1) * VS]),
            "wih": wihA, "whh": whhA, "xq": xqA, "h0": h0A,
            "em": em, "sel": sel,
            "onesr": onesr, "ones32": ones32, "onescol": onescol,
            "idxg": idxg, "idxp": idxp,
        })
    return in_maps


def kernel(**inputs):
    global _NC_CACHE, LAST_RESULTS
    if _NC_CACHE is None:
        _NC_CACHE = _build()
    nc = _NC_CACHE

    in_maps = prepare_in_maps(**inputs)
    res = run_bass_kernel_spmd(nc, in_maps, core_ids=list(range(NCORES)),
                               trace=PROFILE)
    LAST_RESULTS = res

    prob_lg = np.concatenate([np.asarray(res.results[k]["prob_lg"])
                              for k in range(NCORES)], axis=0)
    p_vocab = np.concatenate([np.asarray(res.results[k]["p_vocab"])
                              for k in range(NCORES)], axis=1)
    hidden = np.asarray(res.results[0]["hidden"])[None]
    return prob_lg.astype(np.float32), p_vocab.astype(np.float32), \
        hidden.astype(np.float32)


# revision 65
# speedup vs baseline: 549.3934x; 542.1326x over previous
"""Trainium2 Bass kernel for nn_DecoderMemNN (MemN2N decoder).

Strategy (8 NeuronCores, SPMD single NEFF, all math in exact fp32):
 - Batch-parallel over B=32 (4 batches per core) for the memory-hop loop.
 - Per batch: 4 t-major dma_gathers of 3KB rows from a host-fused table
   F=[C0|C1|C2] at story indices (t=0 lands directly in the m_A tile); the
   T-sum is a chain of DVE/GPSIMD tensor-adds plus the positional-encoding
   rows (their own 1KB-row gather), giving m_A [128, 8, 768] (m = 128*jt+p).
   C3 is dead code (its o_k feeds nothing any output needs) and is never
   touched.
 - Hop h: logits by DVE mult (m_A slice x partition-broadcast u) with the
   row-reduce split DVE/ACT; softmax without max-subtraction (logits are
   small); o_k in COLUMN form - 16 PE matmuls with the m_C slice as the
   stationary operand and the exp vector moving (avoids the fp32 4-cycle
   stream penalty); u lives in PSUM and accumulates (1/total)*o_k via a
   broadcast-stationary matmul, folding the softmax normalization in.
 - p_vocab: the u0 half + bias of the column-parallel W1^T [513, 4000] shard
   runs during the gather phase; o_k(hop0) columns are AllGather'd across
   cores and the okT half overlaps the remaining hops; host concatenates the
   8 vocab shards.
 - GRU + hidden computed on every core (tiny, replicated) in transposed
   layout with biases folded in as an extra contraction row.

kernel(**inputs) takes FULL inputs (as reference.setup_inputs()) and returns
(prob_lg, p_vocab, hidden) like reference.reference().
"""

import numpy as np

import concourse.bacc as bacc
import concourse.mybir as mybir
import concourse.tile as tile
from concourse.bass_utils import run_bass_kernel_spmd

# problem constants (hardcoded per the harness contract)
V = 32000
D = 256
M = 1024
B = 32
T = 4
NCORES = 8
BPC = B // NCORES          # batches per core = 4
VS = V // NCORES           # vocab shard = 4000
E3 = 3 * D                 # fused row width = 768
NBN = 8                    # p_vocab column blocks per core
NBW = VS // NBN            # 500 columns per matmul
F32 = mybir.dt.float32
I16 = mybir.dt.int16

# set by test harness to collect a profile
PROFILE = False
LAST_RESULTS = None


def _wrap_idx(idx_flat: np.ndarray) -> np.ndarray:
    """int16 index list -> [128, n/16] wrapped layout (elem i at partition i%16,
    col i//16), replicated across the 8 gpsimd core groups."""
    n = idx_flat.shape[0]
    w = idx_flat.reshape(n // 16, 16).T.astype(np.int16)   # [16, n/16]
    return np.tile(w, (8, 1)).copy()


def _build(num_devices=NCORES, with_collective=True, work_reps=1):
    nc = bacc.Bacc("TRN2", target_bir_lowering=False, debug=False,
                   num_devices=num_devices)

    # ---- DRAM I/O ----
    F_d = nc.dram_tensor("F", [V, E3], F32, kind="ExternalInput")
    pos_d = nc.dram_tensor("pos", [1025, D], F32, kind="ExternalInput")
    w1t_d = nc.dram_tensor("w1t", [513, VS], F32, kind="ExternalInput")
    wih_d = nc.dram_tensor("wih", [384, 3 * D], F32, kind="ExternalInput")
    whh_d = nc.dram_tensor("whh", [384, 3 * D], F32, kind="ExternalInput")
    xq_d = nc.dram_tensor("xq", [384, B], F32, kind="ExternalInput")
    h0_d = nc.dram_tensor("h0", [384, B], F32, kind="ExternalInput")
    em_d = nc.dram_tensor("em", [128, 128], F32, kind="ExternalInput")
    sel_d = nc.dram_tensor("sel", [32, BPC, 128], F32, kind="ExternalInput")
    onesr_d = nc.dram_tensor("onesr", [1, 128], F32, kind="ExternalInput")
    ones32_d = nc.dram_tensor("ones32", [1, 32], F32, kind="ExternalInput")
    onescol_d = nc.dram_tensor("onescol", [128, 1], F32, kind="ExternalInput")
    idxg_d = nc.dram_tensor("idxg", [128, BPC, 4, 64], I16, kind="ExternalInput")
    idxp_d = nc.dram_tensor("idxp", [128, BPC, 64], I16, kind="ExternalInput")

    plg_d = nc.dram_tensor("prob_lg", [BPC, M], F32, kind="ExternalOutput")
    pv_d = nc.dram_tensor("p_vocab", [B, VS], F32, kind="ExternalOutput")
    hid_d = nc.dram_tensor("hidden", [B, D], F32, kind="ExternalOutput")

    ADD = mybir.AluOpType.add
    MUL = mybir.AluOpType.mult
    SUB = mybir.AluOpType.subtract
    AX = mybir.AxisListType.X
    SIG = mybir.ActivationFunctionType.Sigmoid
    TANH = mybir.ActivationFunctionType.Tanh
    EXP = mybir.ActivationFunctionType.Exp

    with tile.TileContext(nc) as tc:
        with (
            tc.tile_pool(name="const", bufs=1) as cpool,
            tc.tile_pool(name="work", bufs=1) as wpool,
            tc.tile_pool(name="gbuf", bufs=3) as gpool,
            tc.tile_pool(name="mabuf", bufs=3) as mapool,
            tc.tile_pool(name="small", bufs=2) as spool,
            tc.tile_pool(name="w3", bufs=3) as w3pool,
            tc.tile_pool(name="psB", bufs=2, space="PSUM") as psB,
            tc.tile_pool(name="psC", bufs=2, space="PSUM") as psC,
            tc.tile_pool(name="dram", bufs=1, space="DRAM") as dpool,
        ):
            # ---- load constants (index tables first: gathers need them) ----
            idxg_t = cpool.tile([128, BPC, 4, 64], I16)
            nc.sync.dma_start(idxg_t[:], idxg_d[:])
            idxp_t = cpool.tile([128, BPC, 64], I16)
            nc.sync.dma_start(idxp_t[:], idxp_d[:])
            em_t = cpool.tile([128, 128], F32)
            nc.sync.dma_start(em_t[:], em_d[:])
            sel_t = cpool.tile([32, BPC, 128], F32)
            nc.sync.dma_start(sel_t[:], sel_d[:])
            onesr_t = cpool.tile([1, 128], F32)
            nc.sync.dma_start(onesr_t[:], onesr_d[:])
            ones32_t = cpool.tile([1, 32], F32)
            nc.sync.dma_start(ones32_t[:], ones32_d[:])
            onescol_t = cpool.tile([128, 1], F32)
            nc.sync.dma_start(onescol_t[:], onescol_d[:])
            wih_t = gpool.tile([128, 3, 3 * D], F32, tag="G")
            nc.sync.dma_start(wih_t[:], wih_d[:].rearrange("(c p) e -> p c e", p=128))
            whh_t = gpool.tile([128, 3, 3 * D], F32, tag="G")
            nc.sync.dma_start(whh_t[:], whh_d[:].rearrange("(c p) e -> p c e", p=128))
            xq_t = cpool.tile([128, 3, B], F32)
            nc.sync.dma_start(xq_t[:], xq_d[:].rearrange("(c p) b -> p c b", p=128))
            h0_t = cpool.tile([128, 3, B], F32)
            nc.sync.dma_start(h0_t[:], h0_d[:].rearrange("(c p) b -> p c b", p=128))

            ident = em_t[:]

            # ---- GRU (transposed layout; all 32 batches) ----
            gi_ps = psC.tile([128, 6, B], F32, tag="pvps")
            gh_ps = psC.tile([128, 6, B], F32, tag="pvps")
            for ps, w_t, x_t in ((gi_ps, wih_t, xq_t), (gh_ps, whh_t, h0_t)):
                for mc in range(6):
                    for kc in range(3):
                        kk = 128 if kc < 2 else 1
                        nc.tensor.matmul(
                            ps[:, mc, :],
                            w_t[0:kk, kc, mc * 128:(mc + 1) * 128],
                            x_t[0:kk, kc, :],
                            start=(kc == 0), stop=(kc == 2),
                        )
            gi_t = wpool.tile([128, 6, B], F32)
            nc.scalar.copy(gi_t[:], gi_ps[:])
            rz_t = wpool.tile([128, 4, B], F32)
            nc.vector.tensor_tensor(rz_t[:], gi_t[:, 0:4, :], gh_ps[:, 0:4, :], ADD)
            r_t = wpool.tile([128, 2, B], F32)
            nc.scalar.activation(r_t[:], rz_t[:, 0:2, :], SIG)
            z_t = wpool.tile([128, 2, B], F32)
            nc.scalar.activation(z_t[:], rz_t[:, 2:4, :], SIG)
            t3_t = wpool.tile([128, 2, B], F32)
            nc.vector.tensor_tensor(t3_t[:], r_t[:], gh_ps[:, 4:6, :], MUL)
            t4_t = wpool.tile([128, 2, B], F32)
            nc.vector.tensor_tensor(t4_t[:], t3_t[:], gi_t[:, 4:6, :], ADD)
            n_t = wpool.tile([128, 2, B], F32)
            nc.scalar.activation(n_t[:], t4_t[:], TANH)
            d_t = wpool.tile([128, 2, B], F32)
            nc.vector.tensor_tensor(d_t[:], h0_t[:, 0:2, :], n_t[:], SUB)
            zd_t = wpool.tile([128, 2, B], F32)
            nc.vector.tensor_tensor(zd_t[:], z_t[:], d_t[:], MUL)
            hn_t = wpool.tile([128, 2, B], F32)   # h_new^T = u0^T, all 32 batches
            nc.vector.tensor_tensor(hn_t[:], n_t[:], zd_t[:], ADD)

            # h_new rows [32, 256] (hidden output + u0 broadcast source)
            hr_ps = psB.tile([32, 2, 128], F32, tag="sps")
            for c in range(2):
                nc.tensor.matmul(hr_ps[:, c, :], hn_t[:, c, :], ident,
                                 start=True, stop=True)
            hr_t = wpool.tile([32, 2, 128], F32)
            nc.scalar.copy(hr_t[:], hr_ps[:])
            nc.sync.dma_start(hid_d[:], hr_t[:].rearrange("b c p -> b (c p)"))
            hrows = hr_t[:].rearrange("b c p -> b (c p)")  # [32, 256]

            # ---- W1 early half: p_vocab partial from u0 (=h_new) + bias ----
            # (runs during the gather phase; the okT half lands after the
            #  collective and overlaps the remaining hops)
            hn_r = wpool.tile([128, 2, B], F32)
            nc.scalar.copy(hn_r[:], hn_t[:])
            pvu_t = wpool.tile([32, NBN, NBW], F32)
            for nb in range(NBN):
                wku_t = spool.tile([128, 2, NBW], F32, tag="wku")
                nc.sync.dma_start(
                    wku_t[:],
                    w1t_d[0:D, nb * NBW:(nb + 1) * NBW]
                    .rearrange("(c p) n -> p c n", p=128))
                wb_t = spool.tile([1, NBW], F32, tag="wb")
                nc.sync.dma_start(wb_t[:], w1t_d[512:513, nb * NBW:(nb + 1) * NBW])
                pvu_ps = psC.tile([32, NBW], F32, tag="pvps")
                for kc in range(2):
                    nc.tensor.matmul(pvu_ps[:], hn_r[:, kc, :],
                                     wku_t[:, kc, :],
                                     start=(kc == 0), stop=False)
                nc.tensor.matmul(pvu_ps[:], ones32_t[:], wb_t[:],
                                 start=False, stop=True)
                nc.scalar.copy(pvu_t[:, nb, :], pvu_ps[:])

            okin_t = dpool.tile([BPC, 2, 128], F32)
            wko_pre = []
            okall_t = dpool.tile([NCORES, BPC, 2, 128], F32)

            # ---- per local batch ----
            for i0 in range(BPC * work_reps):
                i = i0 % BPC
                # u lives in PSUM; u0 = sel_i^T @ hrows, later hops
                # accumulate broadcast(o_k) into the same bank.
                u_ps = psB.tile([128, D], F32, tag="ups")
                nc.tensor.matmul(u_ps[:], sel_t[:, i, :], hrows,
                                 start=True, stop=False,
                                 skip_group_check=True)

                posg_t = wpool.tile([128, 8, D], F32, tag="posg")
                nc.gpsimd.dma_gather(posg_t[:], pos_d[:], idxp_t[:, i, :],
                                     M, M, D)

                # t-major gathers: gather qt holds rows (t=qt, all 1024 m)
                # as [128, 8, 768] with m = 128*jt + p.  m_A = sum_t + pos.
                # t=0 gathers straight into the mA tile; pos adds in early.
                mA_t = mapool.tile([128, 8, E3], F32, tag="mA")
                nc.gpsimd.dma_gather(mA_t[:], F_d[:], idxg_t[:, i, 0, :],
                                     M, M, E3)
                nc.vector.tensor_tensor(mA_t[:, :, 0:D], mA_t[:, :, 0:D],
                                        posg_t[:], ADD)
                for qt in range(1, 4):
                    g_t = gpool.tile([128, 8, E3], F32, tag="G")
                    nc.gpsimd.dma_gather(g_t[:], F_d[:], idxg_t[:, i, qt, :],
                                         M, M, E3)
                    eng = nc.gpsimd if (qt == 2 and i < BPC - 1) else nc.vector
                    eng.tensor_tensor(mA_t[:], mA_t[:], g_t[:], ADD)

                # ---- hops ----
                for h in range(3):
                    # in-place: mA slice h is dead after this hop's logits
                    prod = mA_t[:, :, h * D:(h + 1) * D]
                    u_b = u_ps[:].rearrange("p (o d) -> p o d", o=1) \
                        .broadcast_to([128, 8, D])
                    nc.vector.tensor_tensor(prod, prod, u_b, MUL)
                    # row-reduce split: DVE takes jt 0-3 in one op, ACT takes
                    # jt 4-7 (copy+accum) in parallel
                    logits_t = spool.tile([128, 8], F32, tag="logits")
                    nc.vector.tensor_reduce(
                        logits_t[:, 0:6], mA_t[:, 0:6, h * D:(h + 1) * D],
                        AX, ADD)
                    for jt in range(6, 8):
                        pslice = mA_t[:, jt, h * D:(h + 1) * D]
                        nc.scalar.activation(
                            pslice, pslice,
                            mybir.ActivationFunctionType.Copy,
                            accum_out=logits_t[:, jt:jt + 1])

                    if h == 2:
                        nc.sync.dma_start(
                            plg_d[i].rearrange("(j p) -> p j", p=128),
                            logits_t[:])
                        continue

                    # softmax with normalization folded into the o_k scale:
                    # o_k = (sum_m exp(l_m) mC_m) / (sum_m exp(l_m))
                    exp_t = spool.tile([128, 8], F32, tag="exp")
                    rsum_t = spool.tile([128, 1], F32, tag="rsum")
                    nc.scalar.activation(exp_t[:], logits_t[:], EXP,
                                         accum_out=rsum_t[:])
                    # 1/total path (parallel with the o_k matmuls below)
                    tot_ps = psB.tile([1, 1], F32, tag="sps")
                    nc.tensor.matmul(tot_ps[:], rsum_t[:], onescol_t[:],
                                     start=True, stop=True)
                    tot_t = spool.tile([1, 1], F32, tag="tot")
                    nc.scalar.copy(tot_t[:], tot_ps[:])
                    totb_ps = psB.tile([128, 1], F32, tag="sps")
                    nc.tensor.matmul(totb_ps[:], onesr_t[:], tot_t[:],
                                     start=True, stop=True)
                    rinv_t = spool.tile([128, 1], F32, tag="rinv")
                    nc.vector.reciprocal(rinv_t[:], totb_ps[:])
                    idtot_t = spool.tile([128, 128], F32, tag="idtot")
                    nc.vector.tensor_scalar_mul(idtot_t[:], ident, rinv_t[:])

                    # o_k in COLUMN form: the mC slice is the stationary
                    # operand (no fp32 4-cycle stream penalty), exp moves N=1
                    okT_ps = psB.tile([128, 2, 1], F32, tag="okps")
                    for c in range(2):
                        for jt in range(8):
                            nc.tensor.matmul(
                                okT_ps[:, c, :],
                                mA_t[:, jt,
                                     (h + 1) * D + c * 128:(h + 1) * D + (c + 1) * 128],
                                exp_t[:, jt:jt + 1],
                                start=(jt == 0), stop=(jt == 7))
                    okT_sb = spool.tile([128, 2, 1], F32, tag="oksb")
                    nc.scalar.copy(okT_sb[:], okT_ps[:])
                    if h == 0:
                        oksc_t = spool.tile([128, 2, 1], F32, tag="oksc")
                        nc.vector.tensor_scalar_mul(oksc_t[:], okT_sb[:],
                                                    rinv_t[:])
                        nc.sync.dma_start(okin_t[i].rearrange("c p -> p c"), oksc_t[:])
                    # u[:, c-chunk] += okT_c (broadcast along free) @ ident/tot
                    for c in range(2):
                        okb = okT_sb[:, c, :].broadcast_to([128, 128])
                        nc.tensor.matmul(u_ps[:, c * 128:(c + 1) * 128],
                                         okb, idtot_t[:],
                                         start=False, stop=(h == 1),
                                         skip_group_check=True)

                if i == 1:
                    # prefetch first 3 okT-half W1 tiles while pool is idle
                    for nb in range(3):
                        wt = w3pool.tile([128, 2, NBW], F32, tag="wko")
                        nc.gpsimd.dma_start(
                            wt[:],
                            w1t_d[D:2 * D, nb * NBW:(nb + 1) * NBW]
                            .rearrange("(c p) n -> p c n", p=128))
                        wko_pre.append(wt)

            # (wko preloads were emitted inside the batch loop)
            # ---- AllGather o_k(hop0) columns; W1 okT half ----
            if with_collective:
                nc.gpsimd.collective_compute(
                    "AllGather", mybir.AluOpType.bypass,
                    replica_groups=[list(range(num_devices))],
                    ins=[okin_t.opt()], outs=[okall_t.opt()])
            else:
                # timing-model variant: local stand-in for the AllGather
                nc.sync.dma_start(okall_t[0], okin_t[:])
            okT_t = wpool.tile([128, 2, 32], F32)
            for c in range(2):
                nc.gpsimd.dma_start(
                    okT_t[:, c, :],
                    okall_t[:, :, c, :].rearrange("k i p -> p (k i)"))

            for nb in range(NBN):
                if nb < 3:
                    wko_t = wko_pre[nb]
                else:
                    wko_t = w3pool.tile([128, 2, NBW], F32, tag="wko")
                    nc.gpsimd.dma_start(
                        wko_t[:],
                        w1t_d[D:2 * D, nb * NBW:(nb + 1) * NBW]
                        .rearrange("(c p) n -> p c n", p=128))
                pv_ps = psC.tile([32, NBW], F32, tag="pvps")
                for kc in range(2):
                    nc.tensor.matmul(pv_ps[:], okT_t[:, kc, :],
                                     wko_t[:, kc, :],
                                     start=(kc == 0), stop=(kc == 1))
                pv_t = spool.tile([32, NBW], F32, tag="pv")
                nc.vector.tensor_tensor(pv_t[:], pvu_t[:, nb, :], pv_ps[:], ADD)
                nc.sync.dma_start(pv_d[:, nb * NBW:(nb + 1) * NBW], pv_t[:])

    nc.compile()
    return nc


_NC_CACHE = None


def prepare_in_maps(story, enc_query, last_hidden, C, pos_table, W1_w, W1_b,
                    gru_w_ih, gru_w_hh, gru_b_ih, gru_b_hh):
    story = np.asarray(story)
    enc_query = np.asarray(enc_query)
    last_hidden = np.asarray(last_hidden, dtype=np.float32)
    C = np.asarray(C, dtype=np.float32)
    pos_table = np.asarray(pos_table, dtype=np.float32)
    W1_w = np.asarray(W1_w, dtype=np.float32)
    W1_b = np.asarray(W1_b, dtype=np.float32)
    gru_w_ih = np.asarray(gru_w_ih, dtype=np.float32)
    gru_w_hh = np.asarray(gru_w_hh, dtype=np.float32)
    gru_b_ih = np.asarray(gru_b_ih, dtype=np.float32)
    gru_b_hh = np.asarray(gru_b_hh, dtype=np.float32)

    # ---- host-side index/layout prep (integer work + small reshapes) ----
    s = np.ascontiguousarray(story.transpose(1, 0, 2))       # [B, M, T] int64
    pad_mask = s[:, :, 0] == 0
    positions = np.cumsum(~pad_mask, axis=-1).astype(np.int64)
    positions[pad_mask] = 0                                   # [B, M]

    F = np.ascontiguousarray(
        np.concatenate([C[0], C[1], C[2]], axis=1), dtype=np.float32)

    # augmented GRU operands (bias folded as contraction row 256; pad to 384)
    def aug_w(wT, b):
        out = np.zeros((384, 3 * D), dtype=np.float32)
        out[0:D] = wT
        out[D] = b
        return out

    wihA = aug_w(gru_w_ih.T, gru_b_ih)
    whhA = aug_w(gru_w_hh.T, gru_b_hh)

    def aug_x(xT):
        out = np.zeros((384, B), dtype=np.float32)
        out[0:D] = xT
        out[D] = 1.0
        return out

    xqA = aug_x(C[0][enc_query].T)
    h0A = aug_x(last_hidden[0].T)

    em = np.eye(128, dtype=np.float32)

    onesr = np.ones((1, 128), dtype=np.float32)
    ones32 = np.ones((1, 32), dtype=np.float32)
    onescol = np.ones((128, 1), dtype=np.float32)

    w1t_full = np.concatenate([W1_w.T, W1_b[None, :]], axis=0)  # [513, 32000]

    in_maps = []
    for k in range(NCORES):
        sel = np.zeros((32, BPC, 128), dtype=np.float32)
        for i in range(BPC):
            sel[BPC * k + i, i, :] = 1.0
        idxg = np.zeros((128, BPC, 4, 64), dtype=np.int16)
        idxp = np.zeros((128, BPC, 64), dtype=np.int16)
        for i in range(BPC):
            b = BPC * k + i
            flat = s[b].T.reshape(-1).astype(np.int16)        # j = t*1024+m
            for qt in range(4):
                idxg[:, i, qt, :] = _wrap_idx(flat[qt * M:(qt + 1) * M])
            idxp[:, i, :] = _wrap_idx(positions[b].astype(np.int16))
        in_maps.append({
            "F": F,
            "pos": pos_table,
            "w1t": np.ascontiguousarray(w1t_full[:, k * VS:(k + 1) * VS]),
            "wih": wihA, "whh": whhA, "xq": xqA, "h0": h0A,
            "em": em, "sel": sel,
            "onesr": onesr, "ones32": ones32, "onescol": onescol,
            "idxg": idxg, "idxp": idxp,
        })
    return in_maps


def kernel(**inputs):
    global _NC_CACHE, LAST_RESULTS
    if _NC_CACHE is None:
        _NC_CACHE = _build()
    nc = _NC_CACHE

    in_maps = prepare_in_maps(**inputs)
    res = run_bass_kernel_spmd(nc, in_maps, core_ids=list(range(NCORES)),
                               trace=PROFILE)
    LAST_RESULTS = res

    prob_lg = np.concatenate([np.asarray(res.results[k]["prob_lg"])
                              for k in range(NCORES)], axis=0)
    p_vocab = np.concatenate([np.asarray(res.results[k]["p_vocab"])
                              for k in range(NCORES)], axis=1)
    hidden = np.asarray(res.results[0]["hidden"])[None]
    return prob_lg.astype(np.float32), p_vocab.astype(np.float32), \
        hidden.astype(np.float32)


# revision 69
# speedup vs baseline: 553.0146x; 1.0066x over previous
"""Trainium2 Bass kernel for nn_DecoderMemNN (MemN2N decoder).

Strategy (8 NeuronCores, SPMD single NEFF, all math in exact fp32):
 - Batch-parallel over B=32 (4 batches per core) for the memory-hop loop.
 - Per batch: 4 t-major dma_gathers of 3KB rows from a host-fused table
   F=[C0|C1|C2] at story indices (t=0 lands directly in the m_A tile); the
   T-sum is a chain of DVE/GPSIMD tensor-adds plus the positional-encoding
   rows (their own 1KB-row gather), giving m_A [128, 8, 768] (m = 128*jt+p).
   C3 is dead code (its o_k feeds nothing any output needs) and is never
   touched.
 - Hop h: logits by DVE mult (m_A slice x partition-broadcast u) with the
   row-reduce split DVE/ACT; softmax without max-subtraction (logits are
   small); o_k in COLUMN form - 16 PE matmuls with the m_C slice as the
   stationary operand and the exp vector moving (avoids the fp32 4-cycle
   stream penalty); u lives in PSUM and accumulates (1/total)*o_k via a
   broadcast-stationary matmul, folding the softmax normalization in.
 - p_vocab: the u0 half + bias of the column-parallel W1^T [513, 4000] shard
   runs during the gather phase; o_k(hop0) columns are AllGather'd across
   cores and the okT half overlaps the remaining hops; host concatenates the
   8 vocab shards.
 - GRU + hidden computed on every core (tiny, replicated) in transposed
   layout with biases folded in as an extra contraction row.

kernel(**inputs) takes FULL inputs (as reference.setup_inputs()) and returns
(prob_lg, p_vocab, hidden) like reference.reference().
"""

import numpy as np

import concourse.bacc as bacc
import concourse.mybir as mybir
import concourse.tile as tile
from concourse.bass_utils import run_bass_kernel_spmd

# problem constants (hardcoded per the harness contract)
V = 32000
D = 256
M = 1024
B = 32
T = 4
NCORES = 8
BPC = B // NCORES          # batches per core = 4
VS = V // NCORES           # vocab shard = 4000
E3 = 3 * D                 # fused row width = 768
NBN = 8                    # p_vocab column blocks per core
NBW = VS // NBN            # 500 columns per matmul
F32 = mybir.dt.float32
I16 = mybir.dt.int16

# set by test harness to collect a profile
PROFILE = False
LAST_RESULTS = None


def _wrap_idx(idx_flat: np.ndarray) -> np.ndarray:
    """int16 index list -> [128, n/16] wrapped layout (elem i at partition i%16,
    col i//16), replicated across the 8 gpsimd core groups."""
    n = idx_flat.shape[0]
    w = idx_flat.reshape(n // 16, 16).T.astype(np.int16)   # [16, n/16]
    return np.tile(w, (8, 1)).copy()


def _build(num_devices=NCORES, with_collective=True, work_reps=1):
    nc = bacc.Bacc("TRN2", target_bir_lowering=False, debug=False,
                   num_devices=num_devices)

    # ---- DRAM I/O ----
    F_d = nc.dram_tensor("F", [V, E3], F32, kind="ExternalInput")
    pos_d = nc.dram_tensor("pos", [1025, D], F32, kind="ExternalInput")
    w1t_d = nc.dram_tensor("w1t", [513, VS], F32, kind="ExternalInput")
    wih_d = nc.dram_tensor("wih", [384, 3 * D], F32, kind="ExternalInput")
    whh_d = nc.dram_tensor("whh", [384, 3 * D], F32, kind="ExternalInput")
    xq_d = nc.dram_tensor("xq", [384, B], F32, kind="ExternalInput")
    h0_d = nc.dram_tensor("h0", [384, B], F32, kind="ExternalInput")
    em_d = nc.dram_tensor("em", [128, 128], F32, kind="ExternalInput")
    sel_d = nc.dram_tensor("sel", [32, BPC, 128], F32, kind="ExternalInput")
    onesr_d = nc.dram_tensor("onesr", [1, 128], F32, kind="ExternalInput")
    ones32_d = nc.dram_tensor("ones32", [1, 32], F32, kind="ExternalInput")
    onescol_d = nc.dram_tensor("onescol", [128, 1], F32, kind="ExternalInput")
    idxg_d = nc.dram_tensor("idxg", [128, BPC, 4, 64], I16, kind="ExternalInput")
    idxp_d = nc.dram_tensor("idxp", [128, BPC, 64], I16, kind="ExternalInput")

    plg_d = nc.dram_tensor("prob_lg", [BPC, M], F32, kind="ExternalOutput")
    pv_d = nc.dram_tensor("p_vocab", [B, VS], F32, kind="ExternalOutput")
    hid_d = nc.dram_tensor("hidden", [B, D], F32, kind="ExternalOutput")

    ADD = mybir.AluOpType.add
    MUL = mybir.AluOpType.mult
    SUB = mybir.AluOpType.subtract
    AX = mybir.AxisListType.X
    SIG = mybir.ActivationFunctionType.Sigmoid
    TANH = mybir.ActivationFunctionType.Tanh
    EXP = mybir.ActivationFunctionType.Exp

    with tile.TileContext(nc) as tc:
        with (
            tc.tile_pool(name="const", bufs=1) as cpool,
            tc.tile_pool(name="work", bufs=1) as wpool,
            tc.tile_pool(name="gbuf", bufs=3) as gpool,
            tc.tile_pool(name="mabuf", bufs=3) as mapool,
            tc.tile_pool(name="small", bufs=2) as spool,
            tc.tile_pool(name="w3", bufs=3) as w3pool,
            tc.tile_pool(name="psB", bufs=2, space="PSUM") as psB,
            tc.tile_pool(name="psC", bufs=2, space="PSUM") as psC,
            tc.tile_pool(name="dram", bufs=1, space="DRAM") as dpool,
        ):
            # ---- load constants (index tables first: gathers need them) ----
            idxg_t = cpool.tile([128, BPC, 4, 64], I16)
            nc.sync.dma_start(idxg_t[:], idxg_d[:])
            idxp_t = cpool.tile([128, BPC, 64], I16)
            nc.sync.dma_start(idxp_t[:], idxp_d[:])
            em_t = cpool.tile([128, 128], F32)
            nc.sync.dma_start(em_t[:], em_d[:])
            sel_t = cpool.tile([32, BPC, 128], F32)
            nc.sync.dma_start(sel_t[:], sel_d[:])
            onesr_t = cpool.tile([1, 128], F32)
            nc.sync.dma_start(onesr_t[:], onesr_d[:])
            ones32_t = cpool.tile([1, 32], F32)
            nc.sync.dma_start(ones32_t[:], ones32_d[:])
            onescol_t = cpool.tile([128, 1], F32)
            nc.sync.dma_start(onescol_t[:], onescol_d[:])
            wih_t = gpool.tile([128, 3, 3 * D], F32, tag="G")
            nc.sync.dma_start(wih_t[:], wih_d[:].rearrange("(c p) e -> p c e", p=128))
            whh_t = gpool.tile([128, 3, 3 * D], F32, tag="G")
            nc.sync.dma_start(whh_t[:], whh_d[:].rearrange("(c p) e -> p c e", p=128))
            xq_t = cpool.tile([128, 3, B], F32)
            nc.sync.dma_start(xq_t[:], xq_d[:].rearrange("(c p) b -> p c b", p=128))
            h0_t = cpool.tile([128, 3, B], F32)
            nc.sync.dma_start(h0_t[:], h0_d[:].rearrange("(c p) b -> p c b", p=128))

            ident = em_t[:]

            # ---- GRU (transposed layout; all 32 batches) ----
            gi_ps = psC.tile([128, 6, B], F32, tag="pvps")
            gh_ps = psC.tile([128, 6, B], F32, tag="pvps")
            for ps, w_t, x_t in ((gi_ps, wih_t, xq_t), (gh_ps, whh_t, h0_t)):
                for mc in range(6):
                    for kc in range(3):
                        kk = 128 if kc < 2 else 1
                        nc.tensor.matmul(
                            ps[:, mc, :],
                            w_t[0:kk, kc, mc * 128:(mc + 1) * 128],
                            x_t[0:kk, kc, :],
                            start=(kc == 0), stop=(kc == 2),
                        )
            gi_t = wpool.tile([128, 6, B], F32)
            nc.scalar.copy(gi_t[:], gi_ps[:])
            rz_t = wpool.tile([128, 4, B], F32)
            nc.vector.tensor_tensor(rz_t[:], gi_t[:, 0:4, :], gh_ps[:, 0:4, :], ADD)
            r_t = wpool.tile([128, 2, B], F32)
            nc.scalar.activation(r_t[:], rz_t[:, 0:2, :], SIG)
            z_t = wpool.tile([128, 2, B], F32)
            nc.scalar.activation(z_t[:], rz_t[:, 2:4, :], SIG)
            t3_t = wpool.tile([128, 2, B], F32)
            nc.vector.tensor_tensor(t3_t[:], r_t[:], gh_ps[:, 4:6, :], MUL)
            t4_t = wpool.tile([128, 2, B], F32)
            nc.vector.tensor_tensor(t4_t[:], t3_t[:], gi_t[:, 4:6, :], ADD)
            n_t = wpool.tile([128, 2, B], F32)
            nc.scalar.activation(n_t[:], t4_t[:], TANH)
            d_t = wpool.tile([128, 2, B], F32)
            nc.vector.tensor_tensor(d_t[:], h0_t[:, 0:2, :], n_t[:], SUB)
            zd_t = wpool.tile([128, 2, B], F32)
            nc.vector.tensor_tensor(zd_t[:], z_t[:], d_t[:], MUL)
            hn_t = wpool.tile([128, 2, B], F32)   # h_new^T = u0^T, all 32 batches
            nc.vector.tensor_tensor(hn_t[:], n_t[:], zd_t[:], ADD)

            # h_new rows [32, 256] (hidden output + u0 broadcast source)
            hr_ps = psB.tile([32, 2, 128], F32, tag="sps")
            for c in range(2):
                nc.tensor.matmul(hr_ps[:, c, :], hn_t[:, c, :], ident,
                                 start=True, stop=True)
            hr_t = wpool.tile([32, 2, 128], F32)
            nc.scalar.copy(hr_t[:], hr_ps[:])
            nc.sync.dma_start(hid_d[:], hr_t[:].rearrange("b c p -> b (c p)"))
            hrows = hr_t[:].rearrange("b c p -> b (c p)")  # [32, 256]

            # ---- W1 early half: p_vocab partial from u0 (=h_new) + bias ----
            # (runs during the gather phase; the okT half lands after the
            #  collective and overlaps the remaining hops)
            hn_r = wpool.tile([128, 2, B], F32)
            nc.scalar.copy(hn_r[:], hn_t[:])
            pvu_t = wpool.tile([32, NBN, NBW], F32)
            for nb in range(NBN):
                wku_t = spool.tile([128, 2, NBW], F32, tag="wku")
                nc.sync.dma_start(
                    wku_t[:],
                    w1t_d[0:D, nb * NBW:(nb + 1) * NBW]
                    .rearrange("(c p) n -> p c n", p=128))
                wb_t = spool.tile([1, NBW], F32, tag="wb")
                nc.sync.dma_start(wb_t[:], w1t_d[512:513, nb * NBW:(nb + 1) * NBW])
                pvu_ps = psC.tile([32, NBW], F32, tag="pvps")
                for kc in range(2):
                    nc.tensor.matmul(pvu_ps[:], hn_r[:, kc, :],
                                     wku_t[:, kc, :],
                                     start=(kc == 0), stop=False)
                nc.tensor.matmul(pvu_ps[:], ones32_t[:], wb_t[:],
                                 start=False, stop=True)
                nc.scalar.copy(pvu_t[:, nb, :], pvu_ps[:])

            okin_t = dpool.tile([BPC, 2, 128], F32)
            wko_pre = []
            okall_t = dpool.tile([NCORES, BPC, 2, 128], F32)

            # ---- per local batch ----
            for i0 in range(BPC * work_reps):
                i = i0 % BPC
                # u lives in PSUM; u0 = sel_i^T @ hrows, later hops
                # accumulate broadcast(o_k) into the same bank.
                u_ps = psB.tile([128, D], F32, tag="ups")
                nc.tensor.matmul(u_ps[:], sel_t[:, i, :], hrows,
                                 start=True, stop=False,
                                 skip_group_check=True)

                posg_t = wpool.tile([128, 8, D], F32, tag="posg")
                nc.gpsimd.dma_gather(posg_t[:], pos_d[:], idxp_t[:, i, :],
                                     M, M, D)

                # t-major gathers: gather qt holds rows (t=qt, all 1024 m)
                # as [128, 8, 768] with m = 128*jt + p.  m_A = sum_t + pos.
                # t=0 gathers straight into the mA tile; pos adds in early.
                mA_t = mapool.tile([128, 8, E3], F32, tag="mA")
                nc.gpsimd.dma_gather(mA_t[:], F_d[:], idxg_t[:, i, 0, :],
                                     M, M, E3)
                nc.vector.tensor_tensor(mA_t[:, :, 0:D], mA_t[:, :, 0:D],
                                        posg_t[:], ADD)
                for qt in range(1, 4):
                    g_t = gpool.tile([128, 8, E3], F32, tag="G")
                    nc.gpsimd.dma_gather(g_t[:], F_d[:], idxg_t[:, i, qt, :],
                                         M, M, E3)
                    if qt == 3 and i == BPC - 1:
                        nc.vector.tensor_tensor(mA_t[:, 0:6, :], mA_t[:, 0:6, :],
                                                g_t[:, 0:6, :], ADD)
                        nc.gpsimd.tensor_tensor(mA_t[:, 6:8, :], mA_t[:, 6:8, :],
                                                g_t[:, 6:8, :], ADD)
                    else:
                        eng = nc.gpsimd if (qt == 2 and i < BPC - 1) else nc.vector
                        eng.tensor_tensor(mA_t[:], mA_t[:], g_t[:], ADD)

                # ---- hops ----
                for h in range(3):
                    # in-place: mA slice h is dead after this hop's logits
                    prod = mA_t[:, :, h * D:(h + 1) * D]
                    u_b = u_ps[:].rearrange("p (o d) -> p o d", o=1) \
                        .broadcast_to([128, 8, D])
                    nc.vector.tensor_tensor(prod, prod, u_b, MUL)
                    # row-reduce split: DVE takes jt 0-3 in one op, ACT takes
                    # jt 4-7 (copy+accum) in parallel
                    logits_t = spool.tile([128, 8], F32, tag="logits")
                    nc.vector.tensor_reduce(
                        logits_t[:, 0:6], mA_t[:, 0:6, h * D:(h + 1) * D],
                        AX, ADD)
                    for jt in range(6, 8):
                        pslice = mA_t[:, jt, h * D:(h + 1) * D]
                        nc.scalar.activation(
                            pslice, pslice,
                            mybir.ActivationFunctionType.Copy,
                            accum_out=logits_t[:, jt:jt + 1])

                    if h == 2:
                        nc.sync.dma_start(
                            plg_d[i].rearrange("(j p) -> p j", p=128),
                            logits_t[:])
                        continue

                    # softmax with normalization folded into the o_k scale:
                    # o_k = (sum_m exp(l_m) mC_m) / (sum_m exp(l_m))
                    exp_t = spool.tile([128, 8], F32, tag="exp")
                    rsum_t = spool.tile([128, 1], F32, tag="rsum")
                    nc.scalar.activation(exp_t[:], logits_t[:], EXP,
                                         accum_out=rsum_t[:])
                    # 1/total path (parallel with the o_k matmuls below)
                    tot_ps = psB.tile([1, 1], F32, tag="sps")
                    nc.tensor.matmul(tot_ps[:], rsum_t[:], onescol_t[:],
                                     start=True, stop=True)
                    tot_t = spool.tile([1, 1], F32, tag="tot")
                    nc.scalar.copy(tot_t[:], tot_ps[:])
                    totb_ps = psB.tile([128, 1], F32, tag="sps")
                    nc.tensor.matmul(totb_ps[:], onesr_t[:], tot_t[:],
                                     start=True, stop=True)
                    rinv_t = spool.tile([128, 1], F32, tag="rinv")
                    nc.vector.reciprocal(rinv_t[:], totb_ps[:])
                    idtot_t = spool.tile([128, 128], F32, tag="idtot")
                    nc.vector.tensor_scalar_mul(idtot_t[:], ident, rinv_t[:])

                    # o_k in COLUMN form: the mC slice is the stationary
                    # operand (no fp32 4-cycle stream penalty), exp moves N=1
                    okT_ps = psB.tile([128, 2, 1], F32, tag="okps")
                    for c in range(2):
                        for jt in range(8):
                            nc.tensor.matmul(
                                okT_ps[:, c, :],
                                mA_t[:, jt,
                                     (h + 1) * D + c * 128:(h + 1) * D + (c + 1) * 128],
                                exp_t[:, jt:jt + 1],
                                start=(jt == 0), stop=(jt == 7))
                    okT_sb = spool.tile([128, 2, 1], F32, tag="oksb")
                    nc.scalar.copy(okT_sb[:], okT_ps[:])
                    if h == 0:
                        oksc_t = spool.tile([128, 2, 1], F32, tag="oksc")
                        nc.vector.tensor_scalar_mul(oksc_t[:], okT_sb[:],
                                                    rinv_t[:])
                        nc.sync.dma_start(okin_t[i].rearrange("c p -> p c"), oksc_t[:])
                    # u[:, c-chunk] += okT_c (broadcast along free) @ ident/tot
                    for c in range(2):
                        okb = okT_sb[:, c, :].broadcast_to([128, 128])
                        nc.tensor.matmul(u_ps[:, c * 128:(c + 1) * 128],
                                         okb, idtot_t[:],
                                         start=False, stop=(h == 1),
                                         skip_group_check=True)

                if i == 1:
                    # prefetch first 3 okT-half W1 tiles while pool is idle
                    for nb in range(3):
                        wt = w3pool.tile([128, 2, NBW], F32, tag="wko")
                        nc.gpsimd.dma_start(
                            wt[:],
                            w1t_d[D:2 * D, nb * NBW:(nb + 1) * NBW]
                            .rearrange("(c p) n -> p c n", p=128))
                        wko_pre.append(wt)

            # (wko preloads were emitted inside the batch loop)
            # ---- AllGather o_k(hop0) columns; W1 okT half ----
            if with_collective:
                nc.gpsimd.collective_compute(
                    "AllGather", mybir.AluOpType.bypass,
                    replica_groups=[list(range(num_devices))],
                    ins=[okin_t.opt()], outs=[okall_t.opt()])
            else:
                # timing-model variant: local stand-in for the AllGather
                nc.sync.dma_start(okall_t[0], okin_t[:])
            okT_t = wpool.tile([128, 2, 32], F32)
            for c in range(2):
                nc.gpsimd.dma_start(
                    okT_t[:, c, :],
                    okall_t[:, :, c, :].rearrange("k i p -> p (k i)"))

            for nb in range(NBN):
                if nb < 3:
                    wko_t = wko_pre[nb]
                else:
                    wko_t = w3pool.tile([128, 2, NBW], F32, tag="wko")
                    nc.gpsimd.dma_start(
                        wko_t[:],
                        w1t_d[D:2 * D, nb * NBW:(nb + 1) * NBW]
                        .rearrange("(c p) n -> p c n", p=128))
                pv_ps = psC.tile([32, NBW], F32, tag="pvps")
                for kc in range(2):
                    nc.tensor.matmul(pv_ps[:], okT_t[:, kc, :],
                                     wko_t[:, kc, :],
                                     start=(kc == 0), stop=(kc == 1))
                pv_t = spool.tile([32, NBW], F32, tag="pv")
                nc.vector.tensor_tensor(pv_t[:], pvu_t[:, nb, :], pv_ps[:], ADD)
                nc.sync.dma_start(pv_d[:, nb * NBW:(nb + 1) * NBW], pv_t[:])

    nc.compile()
    return nc


_NC_CACHE = None


def prepare_in_maps(story, enc_query, last_hidden, C, pos_table, W1_w, W1_b,
                    gru_w_ih, gru_w_hh, gru_b_ih, gru_b_hh):
    story = np.asarray(story)
    enc_query = np.asarray(enc_query)
    last_hidden = np.asarray(last_hidden, dtype=np.float32)
    C = np.asarray(C, dtype=np.float32)
    pos_table = np.asarray(pos_table, dtype=np.float32)
    W1_w = np.asarray(W1_w, dtype=np.float32)
    W1_b = np.asarray(W1_b, dtype=np.float32)
    gru_w_ih = np.asarray(gru_w_ih, dtype=np.float32)
    gru_w_hh = np.asarray(gru_w_hh, dtype=np.float32)
    gru_b_ih = np.asarray(gru_b_ih, dtype=np.float32)
    gru_b_hh = np.asarray(gru_b_hh, dtype=np.float32)

    # ---- host-side index/layout prep (integer work + small reshapes) ----
    s = np.ascontiguousarray(story.transpose(1, 0, 2))       # [B, M, T] int64
    pad_mask = s[:, :, 0] == 0
    positions = np.cumsum(~pad_mask, axis=-1).astype(np.int64)
    positions[pad_mask] = 0                                   # [B, M]

    F = np.ascontiguousarray(
        np.concatenate([C[0], C[1], C[2]], axis=1), dtype=np.float32)

    # augmented GRU operands (bias folded as contraction row 256; pad to 384)
    def aug_w(wT, b):
        out = np.zeros((384, 3 * D), dtype=np.float32)
        out[0:D] = wT
        out[D] = b
        return out

    wihA = aug_w(gru_w_ih.T, gru_b_ih)
    whhA = aug_w(gru_w_hh.T, gru_b_hh)

    def aug_x(xT):
        out = np.zeros((384, B), dtype=np.float32)
        out[0:D] = xT
        out[D] = 1.0
        return out

    xqA = aug_x(C[0][enc_query].T)
    h0A = aug_x(last_hidden[0].T)

    em = np.eye(128, dtype=np.float32)

    onesr = np.ones((1, 128), dtype=np.float32)
    ones32 = np.ones((1, 32), dtype=np.float32)
    onescol = np.ones((128, 1), dtype=np.float32)

    w1t_full = np.concatenate([W1_w.T, W1_b[None, :]], axis=0)  # [513, 32000]

    in_maps = []
    for k in range(NCORES):
        sel = np.zeros((32, BPC, 128), dtype=np.float32)
        for i in range(BPC):
            sel[BPC * k + i, i, :] = 1.0
        idxg = np.zeros((128, BPC, 4, 64), dtype=np.int16)
        idxp = np.zeros((128, BPC, 64), dtype=np.int16)
        for i in range(BPC):
            b = BPC * k + i
            flat = s[b].T.reshape(-1).astype(np.int16)        # j = t*1024+m
            for qt in range(4):
                idxg[:, i, qt, :] = _wrap_idx(flat[qt * M:(qt + 1) * M])
            idxp[:, i, :] = _wrap_idx(positions[b].astype(np.int16))
        in_maps.append({
            "F": F,
            "pos": pos_table,
            "w1t": np.ascontiguousarray(w1t_full[:, k * VS:(k + 1) * VS]),
            "wih": wihA, "whh": whhA, "xq": xqA, "h0": h0A,
            "em": em, "sel": sel,
            "onesr": onesr, "ones32": ones32, "onescol": onescol,
            "idxg": idxg, "idxp": idxp,
        })
    return in_maps


def kernel(**inputs):
    global _NC_CACHE, LAST_RESULTS
    if _NC_CACHE is None:
        _NC_CACHE = _build()
    nc = _NC_CACHE

    in_maps = prepare_in_maps(**inputs)
    res = run_bass_kernel_spmd(nc, in_maps, core_ids=list(range(NCORES)),
                               trace=PROFILE)
    LAST_RESULTS = res

    prob_lg = np.concatenate([np.asarray(res.results[k]["prob_lg"])
                              for k in range(NCORES)], axis=0)
    p_vocab = np.concatenate([np.asarray(res.results[k]["p_vocab"])
                              for k in range(NCORES)], axis=1)
    hidden = np.asarray(res.results[0]["hidden"])[None]
    return prob_lg.astype(np.float32), p_vocab.astype(np.float32), \
        hidden.astype(np.float32)
